# revision 21
# baseline (speedup 1.0000x reference)
"""ConvGRU Trainium2 kernel.

video [B=2, T=16, C=128, H=64, W=64] f32; 1x1-conv GRU over T.
Sharding: data-parallel over (B x H/16) -> 8 cores, each core owns
P = 16*64 = 1024 pixels for all T; weights replicated.

Per core, per timestep (pixels on the free dim, channels on partitions),
two pixel groups a/b (fixed roles, b trails a by ~half a period) pipeline
the serial recurrence:
    r  = sigmoid(Wrx@x + Wrh@h + br)        (PE -> ACT)
    zb = sigmoid(-(Wzx@x + Wzh@h + bz))     (PE -> ACT)   zb = 1-z
    rh = r * h                              (DVE TT)
    c  = tanh(Whx@x + Whh@rh + bh)          (PE -> ACT)
    z  = 1 - zb                             (DVE TS, 4x mode)
    u  = zb * h                             (POOL for a, DVE for b)
    v  = z * c ; h' = u + v                 (DVE TT, chain tail)

The step period (~4.3us) is paced by the in-order ACT queue (6 ops,
~4.1us busy); issue order everywhere is chosen so ops become ready just
as their queue reaches them:
  ACT: sr(a), szb(a), sr(b), tanh(a), szb(b), tanh(b)
  DVE: rh(a), rh(b), z(a), v(a), h'(a), u(b), z(b), v(b), h'(b)
  PE : rcl(a), zopen(b,t), zcl(a), rcl(b), zcl(b), copen(a/b,t+1),
       ccl(a), ropen(a,t+1), ccl(b), ropen(b,t+1), zopen(a,t+1)
       [zopen(b,t+1) deferred to the next iteration so the t+1 closes
        aren't stuck behind it at the step boundary]

Startup: a [C,1] dummy sigmoid pulls the ~2.7us ACT table load to t~0
and garbage matmuls ramp the PE clock, both overlapping the weight/x0
DMAs; only the lead group's zr openers precede the loop so the first
r-close issues as soon as x0 lands.

Things measured slower on TRN2 (don't "fix" these back in):
  - scalar_tensor_tensor has no fp16 perf mode (~760ns vs ~420ns TT):
    fusing the tail into STTs loses more than the saved semaphore hop.
  - gpsimd TT is ~1.2us and its queue's DMA dispatch/sem waits block
    later Pool ops: only u(a) (genuinely slack) lives there.
  - merging sigmoid/tanh ops across groups couples the two recurrence
    chains and adds more latency than the saved 352-cycle ACT pipe fill.

Numerics: fp16 matmul inputs/gates/state, fp32 PSUM accum + fp32 bias.
"""

import os
import sys

import numpy as np

B, T, C, H, W = 2, 16, 128, 64, 64
NCORES = 8
HQ = H // 4          # 16 rows of H per core (4 H-slices x 2 batches = 8 cores)
P = HQ * W           # 1024 pixels per core
G = 2                # pixel groups per step (independent recurrence chains)
PG = P // G          # 512 pixels per group

_PROG = None


def _ensure_paths():
    for p in ("/opt/trn_rl_repo",):
        if p not in sys.path and os.path.isdir(p):
            sys.path.append(p)


def _build():
    _ensure_paths()
    import concourse.bacc as bacc
    import concourse.tile as tile
    from concourse import mybir

    f32 = mybir.dt.float32
    f16 = mybir.dt.float16
    AF = mybir.ActivationFunctionType
    ALU = mybir.AluOpType

    nc = bacc.Bacc(
        "TRN2", target_bir_lowering=False, debug=False, num_devices=NCORES
    )
    x_dram = nc.dram_tensor("x_seq", [T, C, P], f16, kind="ExternalInput")
    w_dram = nc.dram_tensor("wmats", [C, 6 * C], f16, kind="ExternalInput")
    b_dram = nc.dram_tensor("biases", [C, 4], f32, kind="ExternalInput")
    o_dram = nc.dram_tensor("out_seq", [T, C, P], f16, kind="ExternalOutput")

    x_ap = x_dram.ap()
    w_ap = w_dram.ap()
    b_ap = b_dram.ap()
    o_ap = o_dram.ap()

    WZX, WZH, WRX, WRH, WHX, WHH = range(6)
    BR, BH, NBZ = 0, 1, 2  # bias columns: br, bh, -bz

    A, Bg = 0, 1  # fixed group roles: a leads, b trails

    with tile.TileContext(nc) as tc:
        with (
            tc.tile_pool(name="consts", bufs=1) as consts,
            tc.tile_pool(name="xin", bufs=4) as xpool,
            tc.tile_pool(name="state", bufs=3) as spool,
            tc.tile_pool(name="work", bufs=3) as wk,
            tc.tile_pool(name="ps", bufs=1, space="PSUM") as ps,
        ):
            # PSUM: z tiles (banks 0-1), r tiles (2-3), c tiles dbl-buf (4-7)
            zp = [
                ps.tile([C, PG], f32, tag=f"z_{g}", bufs=1, name=f"zp{g}")
                for g in range(G)
            ]
            rp = [
                ps.tile([C, PG], f32, tag=f"r_{g}", bufs=1, name=f"rp{g}")
                for g in range(G)
            ]

            # -- warmup, dependency-free so it overlaps the input DMAs:
            #    a [C,1] dummy sigmoid pulls the ACT table load to t~0, and
            #    garbage matmuls ramp the PE clock gate --
            dumm = consts.tile([C, 1], f16)
            nc.vector.memset(dumm[:], 0.0)
            nc.scalar.activation(dumm[:], dumm[:], AF.Sigmoid, bias=0.0)
            junk = consts.tile([C, PG], f16)
            nc.vector.memset(junk[:], 0.0)
            for i in range(4):
                nc.tensor.matmul(
                    zp[0][:], junk[:, :C], junk[:],
                    start=True, stop=True,
                )

            wt = consts.tile([C, 6 * C], f16)
            nc.sync.dma_start(wt[:], w_ap[:])
            bt = consts.tile([C, 4], f32)
            nc.gpsimd.dma_start(bt[:], b_ap[:])

            def wslice(i):
                return wt[:, i * C : (i + 1) * C]

            # fp16 state per pixel group; h(0)=0 so no initial state tile —
            # step 0 skips every h-side op and h16 is first written by its
            # tail (see below)
            h16 = [None] * G

            def load_x(t):
                xt = xpool.tile([C, P], f16, tag="x")
                nc.sync.dma_start(xt[:], x_ap[t])
                return xt

            def xs_of(xt, g):
                return xt[:, g * PG : (g + 1) * PG]

            def ropen(xt, g, full=False):
                nc.tensor.matmul(
                    rp[g][:], wslice(WRX), xs_of(xt, g), start=True, stop=full
                )

            def zopen(xt, g, full=False):
                nc.tensor.matmul(
                    zp[g][:], wslice(WZX), xs_of(xt, g), start=True, stop=full
                )

            def copen(xt, g, full=False):
                cp = ps.tile([C, PG], f32, tag=f"c_{g}", bufs=2, name=f"cp{g}")
                nc.tensor.matmul(
                    cp[:], wslice(WHX), xs_of(xt, g), start=True, stop=full
                )
                return cp

            # only the lead group's openers before the loop: the t=0 PE
            # queue reaches the first r-close with minimal latency, and the
            # trail group's openers slot in behind it. x0 is loaded in
            # per-group halves so the lead half lands ~700ns sooner. h(0)=0,
            # so every t=0 opener is a complete accumulation (stop=True).
            x_t = xpool.tile([C, P], f16, tag="x", name="x0t")
            nc.sync.dma_start(x_t[:, :PG], x_ap[0, :, :PG])
            nc.sync.dma_start(x_t[:, PG:], x_ap[0, :, PG:])
            ropen(x_t, A, full=True)
            zopen(x_t, A, full=True)
            cp_t = None
            v16p = [None] * G  # previous step's v tiles (for split r-close)
            u16p = [None] * G  # previous step's u tiles

            for t in range(T):
                x_next = load_x(t + 1) if t + 1 < T else None

                # -- PE head: split r-closes (Wrh@v; Wrh@u ran during the
                #    previous step) and z-closes; zopen(b,t) deferred here --
                if t > 0:
                    nc.tensor.matmul(
                        rp[A][:], wslice(WRH), v16p[A][:],
                        start=False, stop=True,
                    )
                    zopen(x_t, Bg)
                    nc.tensor.matmul(
                        zp[A][:], wslice(WZH), h16[A][:],
                        start=False, stop=True,
                    )
                    if u16p[Bg] is not None:
                        nc.tensor.matmul(
                            rp[Bg][:], wslice(WRH), u16p[Bg][:],
                            start=False, stop=False,
                        )
                    nc.tensor.matmul(
                        rp[Bg][:], wslice(WRH), v16p[Bg][:],
                        start=False, stop=True,
                    )
                    nc.tensor.matmul(
                        zp[Bg][:], wslice(WZH), h16[Bg][:],
                        start=False, stop=True,
                    )
                else:
                    ropen(x_t, Bg, full=True)
                    zopen(x_t, Bg, full=True)
                    cp_t = [copen(x_t, g, full=True) for g in (A, Bg)]

                # -- ACT: sr(a), szb(a), sr(b) [tanh/szb(b) slotted below] --
                r16, zb16 = [None] * G, [None] * G

                def act_r(g):
                    rt = wk.tile([C, PG], f16, tag=f"r16_{g}", name="rt")
                    nc.scalar.activation(
                        rt[:], rp[g][:], AF.Sigmoid, bias=bt[:, BR : BR + 1]
                    )
                    r16[g] = rt

                def act_zb(g):
                    zbt = wk.tile([C, PG], f16, tag=f"zb_{g}", name="zbt")
                    nc.scalar.activation(
                        zbt[:], zp[g][:], AF.Sigmoid,
                        bias=bt[:, NBZ : NBZ + 1], scale=-1.0,
                    )
                    zb16[g] = zbt

                act_r(A)
                act_zb(A)
                act_r(Bg)

                # -- DVE: rh gates the c matmul (h=0 at t=0: skip) --
                rh16 = [None] * G
                if t > 0:
                    for g in (A, Bg):
                        rh = wk.tile([C, PG], f16, tag=f"rh_{g}", name="rh")
                        nc.vector.tensor_mul(rh[:], r16[g][:], h16[g][:])
                        rh16[g] = rh

                # -- DVE: u(a) = zb*h, free filler while v(a) waits tanh --
                u16 = [None] * G
                if t > 0:
                    ua = wk.tile([C, PG], f16, tag=f"u_{A}", name="ua")
                    nc.vector.tensor_mul(ua[:], zb16[A][:], h16[A][:])
                    u16[A] = ua

                # next step's c openers fill the rh-wait gap on the PE
                cp_next = (
                    [copen(x_next, g) for g in (A, Bg)]
                    if x_next is not None
                    else None
                )

                # -- PE: c closes, t+1 r-openers, and the slack-scheduled
                #    split r-close half Wrh@u(a,t) --
                if t > 0:
                    nc.tensor.matmul(
                        cp_t[A][:], wslice(WHH), rh16[A][:],
                        start=False, stop=True,
                    )
                if x_next is not None:
                    ropen(x_next, A)
                if t > 0:
                    nc.tensor.matmul(
                        cp_t[Bg][:], wslice(WHH), rh16[Bg][:],
                        start=False, stop=True,
                    )
                if x_next is not None:
                    ropen(x_next, Bg)

                # -- ACT: tanh(a); szb(b); tanh(b) --
                c16 = [None] * G

                def act_c(g):
                    ct = wk.tile([C, PG], f16, tag=f"c16_{g}", name="ct")
                    nc.scalar.activation(
                        ct[:], cp_t[g][:], AF.Tanh, bias=bt[:, BH : BH + 1]
                    )
                    c16[g] = ct

                act_c(A)
                act_zb(Bg)
                act_c(Bg)

                # t+1 z-opener for the lead group, then the slack-scheduled
                # split r-close half Wrh@u(a,t); b's z-opener is deferred
                if x_next is not None:
                    zopen(x_next, A)
                    if u16[A] is not None:
                        nc.tensor.matmul(
                            rp[A][:], wslice(WRH), u16[A][:],
                            start=False, stop=False,
                        )

                # -- DVE tail: z = 1-zb (TS, 4x); v = z*c ; h' = u + v.
                #    At t=0 u=0 so h' = v, written straight into the state
                #    tile (which doubles as v for the next r-close). --
                def tail(g):
                    zt = wk.tile([C, PG], f16, tag=f"zt_{g}", name="zt")
                    nc.vector.tensor_scalar(
                        zt[:], zb16[g][:], -1.0, 1.0, ALU.mult, ALU.add
                    )
                    if t == 0:
                        n16 = spool.tile(
                            [C, PG], f16, tag=f"h16_{g}", name="n16"
                        )
                        nc.vector.tensor_mul(n16[:], zt[:], c16[g][:])
                        v16p[g] = n16
                    else:
                        v16 = wk.tile([C, PG], f16, tag=f"v_{g}", name="v16")
                        nc.vector.tensor_mul(v16[:], zt[:], c16[g][:])
                        v16p[g] = v16
                        n16 = spool.tile(
                            [C, PG], f16, tag=f"h16_{g}", name="n16"
                        )
                        nc.vector.tensor_add(n16[:], u16[g][:], v16[:])
                    h16[g] = n16
                    # gpsimd queue is otherwise idle: free dispatch slot
                    nc.gpsimd.dma_start(
                        o_ap[t, :, g * PG : (g + 1) * PG], n16[:]
                    )

                tail(A)

                # -- DVE: u(b) slotted while tanh(b) runs on ACT --
                if t > 0:
                    ub = wk.tile([C, PG], f16, tag=f"u_{Bg}", name="ub")
                    nc.vector.tensor_mul(ub[:], zb16[Bg][:], h16[Bg][:])
                    u16[Bg] = ub

                tail(Bg)

                u16p = u16
                if x_next is not None:
                    x_t, cp_t = x_next, cp_next

    nc.compile()
    return nc


def _get_prog():
    global _PROG
    if _PROG is None:
        _PROG = _build()
    return _PROG


def _make_in_maps(video, Wz, bz, Wr, br, Wh, bh):
    w6 = np.concatenate(
        [
            Wz[:, :C].T, Wz[:, C:].T,
            Wr[:, :C].T, Wr[:, C:].T,
            Wh[:, :C].T, Wh[:, C:].T,
        ],
        axis=1,
    ).astype(np.float16)
    b3 = np.stack([br, bh, -bz, bz], axis=1).astype(np.float32)
    in_maps = []
    for core in range(NCORES):
        b_, q = divmod(core, 4)
        xs = np.ascontiguousarray(
            video[b_, :, :, q * HQ : (q + 1) * HQ, :]
        ).reshape(T, C, P).astype(np.float16)
        in_maps.append({"x_seq": xs, "wmats": w6, "biases": b3})
    return in_maps


def kernel(video, Wz, bz, Wr, br, Wh, bh):
    _ensure_paths()
    from concourse.bass_utils import run_bass_kernel_spmd

    video = np.asarray(video, dtype=np.float32)
    nc = _get_prog()
    in_maps = _make_in_maps(video, Wz, bz, Wr, br, Wh, bh)
    res = run_bass_kernel_spmd(nc, in_maps, list(range(NCORES)))

    out = np.empty((B, T, C, H, W), np.float32)
    for core in range(NCORES):
        b_, q = divmod(core, 4)
        out[b_, :, :, q * HQ : (q + 1) * HQ, :] = np.asarray(
            res.results[core]["out_seq"]
        ).astype(np.float32).reshape(T, C, HQ, W)
    return out
